# revision 1
# baseline (speedup 1.0000x reference)
"""Causal self-attention kernel for 8 TRN2 NeuronCores.

Sharding: 8 cores = 4 batches x 2 head-groups (8 heads / 512 channels each).
Each core computes q/k/v projections for its head half, causal attention for
its 8 heads, and a partial c_proj contracted over its 512 channels. The host
sums the two partials per batch and adds the c_proj bias.

All matmuls run in bf16 with fp32 PSUM accumulation. Host pre-transposes
x and the weight matrices so the device only ever does natural-layout DMAs.
"""

import numpy as np
import ml_dtypes
from contextlib import ExitStack

import concourse.bass as bass
import concourse.tile as tile
from concourse import bacc, mybir
from concourse.bass_utils import run_bass_kernel_spmd

BF16 = mybir.dt.bfloat16
F32 = mybir.dt.float32

N_EMBD = 1024
N_HEAD = 16
B = 4
T_FULL = 2048
HD = 64           # head dim
HPC = 8           # heads per core
CH = HPC * HD     # channels per core = 512
N_CORES = 8
SCALE = 1.0 / 8.0  # 1/sqrt(HD)

P = 128           # partitions
QC = 512          # q-chunk (matmul free dim)


def build_nc(T=T_FULL, pack_qk=True):
    """Build the per-core Bass module (same program on every core)."""
    n_tt = T // P          # 128-row tiles along T
    n_qc = T // QC         # 512-wide chunks along T
    n_ci = N_EMBD // P     # contraction tiles over the full embed dim
    n_dt = CH // P         # d-tiles of this core's 512 channels (= head pairs)

    nc = bacc.Bacc("TRN2", target_bir_lowering=False, debug=False)

    xt = nc.dram_tensor("xt", [N_EMBD, T], BF16, kind="ExternalInput").ap()
    wq = nc.dram_tensor("wq", [N_EMBD, CH], BF16, kind="ExternalInput").ap()
    wk = nc.dram_tensor("wk", [N_EMBD, CH], BF16, kind="ExternalInput").ap()
    wv = nc.dram_tensor("wv", [N_EMBD, CH], BF16, kind="ExternalInput").ap()
    wc = nc.dram_tensor("wc", [CH, N_EMBD], BF16, kind="ExternalInput").ap()
    bq = nc.dram_tensor("bq", [P, n_dt], F32, kind="ExternalInput").ap()
    bk = nc.dram_tensor("bk", [P, n_dt], F32, kind="ExternalInput").ap()
    vb1 = nc.dram_tensor("vb1", [P, HPC * (HD + 1)], F32, kind="ExternalInput").ap()
    masks = nc.dram_tensor("masks", [4, P, QC], BF16, kind="ExternalInput").ap()
    out = nc.dram_tensor("out", [T, N_EMBD], F32, kind="ExternalOutput").ap()

    with tile.TileContext(nc) as tc, ExitStack() as ctx:
        singles = ctx.enter_context(tc.tile_pool(name="singles", bufs=1))
        mm_ps = ctx.enter_context(tc.tile_pool(name="mm_ps", bufs=3, space="PSUM"))
        av_ps_pool = ctx.enter_context(tc.tile_pool(name="av_ps", bufs=3, space="PSUM"))
        pt_pool = ctx.enter_context(tc.tile_pool(name="pt", bufs=8))
        small = ctx.enter_context(tc.tile_pool(name="small", bufs=4))
        dram = ctx.enter_context(tc.tile_pool(name="dram", bufs=4, space="DRAM"))
        ost = ctx.enter_context(tc.tile_pool(name="ost", bufs=3))

        # ---- resident tensors ----
        xt_sb = singles.tile([P, n_ci, T], BF16)
        nc.sync.dma_start(xt_sb, xt.rearrange("(ci p) t -> p ci t", p=P))
        wq_sb = singles.tile([P, n_ci, CH], BF16)
        nc.sync.dma_start(wq_sb, wq.rearrange("(ci p) d -> p ci d", p=P))
        wk_sb = singles.tile([P, n_ci, CH], BF16)
        nc.sync.dma_start(wk_sb, wk.rearrange("(ci p) d -> p ci d", p=P))
        wv_sb = singles.tile([P, n_ci, CH], BF16)
        nc.sync.dma_start(wv_sb, wv.rearrange("(ci p) d -> p ci d", p=P))
        wc_sb = singles.tile([P, n_dt, N_EMBD], BF16)
        nc.sync.dma_start(wc_sb, wc.rearrange("(pr p) o -> p pr o", p=P))
        bq_sb = singles.tile([P, n_dt], F32)
        nc.sync.dma_start(bq_sb, bq)
        bk_sb = singles.tile([P, n_dt], F32)
        nc.sync.dma_start(bk_sb, bk)
        vb1_sb = singles.tile([P, HPC, HD + 1], F32)
        nc.sync.dma_start(vb1_sb, vb1.rearrange("p (h j) -> p h j", h=HPC))
        mask_sb = singles.tile([P, 4, QC], BF16)
        nc.sync.dma_start(mask_sb, masks.rearrange("r p q -> p r q"))

        qt_sb = singles.tile([P, n_dt, T], BF16)   # [d%128, head-pair, t]
        kt_sb = singles.tile([P, n_dt, T], BF16)
        v1_sb = singles.tile([P, n_tt, HPC, HD + 1], BF16)  # [t%128, t//128, h, d|1]
        yt_sb = singles.tile([P, n_dt, T], BF16)   # attention out (normalized)

        # ---- Q^T / K^T projections: out[d, t] ----
        for w_sb, b_sb, o_sb in ((wq_sb, bq_sb, qt_sb), (wk_sb, bk_sb, kt_sb)):
            for dt in range(n_dt):
                for tcn in range(n_qc):
                    ps = mm_ps.tile([P, QC], F32, tag="mm")
                    for ci in range(n_ci):
                        nc.tensor.matmul(
                            ps,
                            lhsT=w_sb[:, ci, dt * P:(dt + 1) * P],
                            rhs=xt_sb[:, ci, tcn * QC:(tcn + 1) * QC],
                            start=(ci == 0), stop=(ci == n_ci - 1),
                        )
                    nc.vector.tensor_add(
                        out=o_sb[:, dt, tcn * QC:(tcn + 1) * QC],
                        in0=ps,
                        in1=b_sb[:, dt, None].to_broadcast((P, QC)),
                    )

        # ---- V projection (natural layout): out[t, d] with ones column ----
        for tt in range(n_tt):
            ps = mm_ps.tile([P, CH], F32, tag="mm")
            for ci in range(n_ci):
                nc.tensor.matmul(
                    ps,
                    lhsT=xt_sb[:, ci, tt * P:(tt + 1) * P],
                    rhs=wv_sb[:, ci, :],
                    start=(ci == 0), stop=(ci == n_ci - 1),
                )
            for h in range(HPC):
                nc.vector.tensor_add(
                    out=v1_sb[:, tt, h, 0:HD],
                    in0=ps[:, h * HD:(h + 1) * HD],
                    in1=vb1_sb[:, h, 0:HD],
                )
            nc.vector.tensor_copy(out=v1_sb[:, tt, :, HD], in_=vb1_sb[:, :, HD])

        # ---- attention, per head pair ----
        for pr in range(n_dt):
            for qc in range(n_qc):
                q0 = qc * QC
                nkt = (q0 + QC) // P  # causal: k-tiles 0..nkt-1
                av_A = av_ps_pool.tile([HD + 1, QC], F32, tag="av")
                av_B = av_ps_pool.tile([HD + 1, QC], F32, tag="av")
                for kt in range(nkt):
                    k0 = kt * P
                    rel = kt - (q0 // P)  # >=0 on diagonal block tiles
                    for hi, (av, prt) in enumerate(((av_A, slice(0, HD)),
                                                    (av_B, slice(HD, P)))):
                        h = 2 * pr + hi
                        qk = mm_ps.tile([P, QC], F32, tag="mm")
                        tp = (prt.start, 0) if pack_qk else None
                        nc.tensor.matmul(
                            qk,
                            lhsT=kt_sb[prt, pr, k0:k0 + P],
                            rhs=qt_sb[prt, pr, q0:q0 + QC],
                            start=True, stop=True,
                            tile_position=tp,
                        )
                        pt = pt_pool.tile([P, QC], BF16, tag="pt")
                        nc.scalar.activation(
                            pt, qk, mybir.ActivationFunctionType.Exp, scale=SCALE,
                        )
                        if rel >= 0:
                            nc.vector.tensor_mul(pt, pt, mask_sb[:, rel, :])
                        nc.tensor.matmul(
                            av,
                            lhsT=v1_sb[:, kt, h, :],
                            rhs=pt,
                            start=(kt == 0), stop=(kt == nkt - 1),
                        )
                # normalize: y[d, q] = av[d, q] / av[HD, q]
                for hi, av in enumerate((av_A, av_B)):
                    r_sb = small.tile([HD + 1, QC], F32, tag="recip")
                    nc.vector.reciprocal(out=r_sb[HD:HD + 1, :], in_=av[HD:HD + 1, :])
                    dscr = dram.tile([1, QC], F32, tag="dbounce")
                    nc.gpsimd.dma_start(out=dscr, in_=r_sb[HD:HD + 1, :])
                    den_sb = small.tile([HD, QC], F32, tag="den")
                    nc.gpsimd.dma_start(
                        out=den_sb,
                        in_=bass.AP(tensor=dscr.tensor, offset=dscr.offset,
                                    ap=[[0, HD]] + list(dscr.ap[1:])),
                    )
                    if hi == 0:
                        nc.vector.tensor_mul(
                            out=yt_sb[0:HD, pr, q0:q0 + QC],
                            in0=av[0:HD, :], in1=den_sb,
                        )
                    else:
                        tmp = small.tile([HD, QC], BF16, tag="ytmp")
                        nc.vector.tensor_mul(out=tmp, in0=av[0:HD, :], in1=den_sb)
                        nc.sync.dma_start(out=yt_sb[HD:P, pr, q0:q0 + QC], in_=tmp)

        # ---- partial c_proj: out[t, :] = sum_pr yt[:, pr, t].T @ wc[pr] ----
        for tt in range(n_tt):
            for oc in range(N_EMBD // QC):
                ps = mm_ps.tile([P, QC], F32, tag="mm")
                for pr in range(n_dt):
                    nc.tensor.matmul(
                        ps,
                        lhsT=yt_sb[:, pr, tt * P:(tt + 1) * P],
                        rhs=wc_sb[:, pr, oc * QC:(oc + 1) * QC],
                        start=(pr == 0), stop=(pr == n_dt - 1),
                    )
                st = ost.tile([P, QC], F32, tag="ost")
                nc.vector.tensor_copy(out=st, in_=ps)
                nc.sync.dma_start(
                    out=out[tt * P:(tt + 1) * P, oc * QC:(oc + 1) * QC], in_=st,
                )

    nc.compile()
    return nc


def make_in_maps(x, Wq, bq, Wk, bk, Wv, bv, T=T_FULL):
    """Host-side sharding + layout prep. Returns per-core input dicts."""
    bf = ml_dtypes.bfloat16
    x = np.asarray(x, dtype=np.float32)
    n_dt = CH // P

    # causal masks for the 4 diagonal-relative offsets
    k_idx = np.arange(P)[:, None]
    q_idx = np.arange(QC)[None, :]
    masks = np.stack(
        [(r * P + k_idx <= q_idx) for r in range(4)]
    ).astype(bf)  # [4, 128, 512]

    wqT = np.ascontiguousarray(Wq.T).astype(bf)  # [cin, dout]
    wkT = np.ascontiguousarray(Wk.T).astype(bf)
    wvT = np.ascontiguousarray(Wv.T).astype(bf)

    in_maps = []
    for core in range(N_CORES):
        b = core // 2
        hh = core % 2
        cs = slice(hh * CH, (hh + 1) * CH)
        xtb = np.ascontiguousarray(x[b, :T].T).astype(bf)  # [N_EMBD, T]

        bq_arr = np.asarray(bq[cs], np.float32).reshape(n_dt, P).T.copy()
        bk_arr = np.asarray(bk[cs], np.float32).reshape(n_dt, P).T.copy()
        vb = np.asarray(bv[cs], np.float32).reshape(HPC, HD)
        vb1 = np.concatenate([vb, np.ones((HPC, 1), np.float32)], axis=1)  # [8, 65]
        vb1 = np.broadcast_to(vb1.reshape(1, -1), (P, HPC * (HD + 1))).copy()

        in_maps.append({
            "xt": xtb,
            "wq": np.ascontiguousarray(wqT[:, cs]),
            "wk": np.ascontiguousarray(wkT[:, cs]),
            "wv": np.ascontiguousarray(wvT[:, cs]),
            "wc": None,  # filled by caller (needs Wc)
            "bq": bq_arr,
            "bk": bk_arr,
            "vb1": vb1,
            "masks": masks,
        })
    return in_maps


_NC_CACHE = {}


def kernel(x, Wq, bq, Wk, bk, Wv, bv, Wc, bc):
    x = np.asarray(x, dtype=np.float32)
    T = x.shape[1]
    key = T
    if key not in _NC_CACHE:
        _NC_CACHE[key] = build_nc(T=T)
    nc = _NC_CACHE[key]

    in_maps = make_in_maps(x, Wq, bq, Wk, bk, Wv, bv, T=T)
    wcT = np.ascontiguousarray(np.asarray(Wc, np.float32).T).astype(
        ml_dtypes.bfloat16)  # [cin, cout]
    for core in range(N_CORES):
        hh = core % 2
        in_maps[core]["wc"] = np.ascontiguousarray(wcT[hh * CH:(hh + 1) * CH, :])

    res = run_bass_kernel_spmd(nc, in_maps, core_ids=list(range(N_CORES)))

    bc = np.asarray(bc, np.float32)
    out = np.empty((B, T, N_EMBD), np.float32)
    for b in range(B):
        out[b] = res.results[2 * b]["out"] + res.results[2 * b + 1]["out"] + bc
    return out


# revision 14
# speedup vs baseline: 1.2521x; 1.2521x over previous
"""Causal self-attention kernel for 8 TRN2 NeuronCores.

Sharding: 8 cores = 4 batches x 2 head-groups (8 heads / 512 channels each).
Each core computes q/k/v projections for its head half, causal attention for
its 8 heads, and a partial c_proj contracted over its 512 channels. The host
sums the two partials per batch and adds the c_proj bias.

All matmuls run in bf16 with fp32 PSUM accumulation. Host pre-transposes
x and the weight matrices so the device only ever does natural-layout DMAs.

Device layouts (per core):
  xt_sb [128, 8, T]        x^T tiles: [cin%128, cin//128, t]
  qt/kt_sb [128, 4, T]     Q^T/K^T: [d%128, head-pair, t] (head 2p: rows 0-63)
  v1_sb [128, T//128, 65, 8]  V interleaved [t%128, t//128, j, h]; col j<64 is
                           V_h[d=j] (host permutes Wv cols to dd*8+h), j=64
                           is the ones column used to accumulate the softmax
                           denominator inside the P@V matmul (M=65).

Scores are computed transposed (S^T[k, q]) so softmax'd probabilities feed
P@V directly as the moving operand; exp runs on ScalarE from PSUM with the
1/sqrt(hd) scale folded in; diagonal blocks are masked post-exp and trimmed
to their valid q-range. The attention inner loop is ScalarE-paced, so
projection / c_proj matmul groups are interleaved into it at k-tile
granularity to keep TensorE busy, and P@V is emitted one k-tile behind the
exp that feeds it.
"""

import numpy as np
import ml_dtypes
from contextlib import ExitStack

import concourse.bass as bass
import concourse.tile as tile
from concourse import bacc, mybir
from concourse.bass_utils import run_bass_kernel_spmd

BF16 = mybir.dt.bfloat16
F32 = mybir.dt.float32

N_EMBD = 1024
N_HEAD = 16
B = 4
T_FULL = 2048
HD = 64           # head dim
HPC = 8           # heads per core
CH = HPC * HD     # channels per core = 512
N_CORES = 8
SCALE = 1.0 / 8.0  # 1/sqrt(HD)

P = 128           # partitions
QC = 512          # q-chunk (matmul free dim)


def build_nc(T=T_FULL, pack_qk=True):
    """Build the per-core Bass module (same program on every core)."""
    n_tt = T // P          # 128-row tiles along T
    n_qc = T // QC         # 512-wide chunks along T
    n_ci = N_EMBD // P     # contraction tiles over the full embed dim
    n_dt = CH // P         # d-tiles of this core's 512 channels (= head pairs)
    n_oc = N_EMBD // QC

    nc = bacc.Bacc("TRN2", target_bir_lowering=False, debug=False)

    xt = nc.dram_tensor("xt", [N_EMBD, T], BF16, kind="ExternalInput").ap()
    wq = nc.dram_tensor("wq", [N_EMBD, CH], BF16, kind="ExternalInput").ap()
    wk = nc.dram_tensor("wk", [N_EMBD, CH], BF16, kind="ExternalInput").ap()
    wv = nc.dram_tensor("wv", [N_EMBD, CH], BF16, kind="ExternalInput").ap()
    wc = nc.dram_tensor("wc", [CH, N_EMBD], BF16, kind="ExternalInput").ap()
    bq = nc.dram_tensor("bq", [P, n_dt], F32, kind="ExternalInput").ap()
    bk = nc.dram_tensor("bk", [P, n_dt], F32, kind="ExternalInput").ap()
    vb1 = nc.dram_tensor("vb1", [P, CH + HPC], F32, kind="ExternalInput").ap()
    masks = nc.dram_tensor("masks", [4, P, QC], BF16, kind="ExternalInput").ap()
    out = nc.dram_tensor("out", [T, N_EMBD], F32, kind="ExternalOutput").ap()

    with tile.TileContext(nc) as tc, ExitStack() as ctx:
        singles = ctx.enter_context(tc.tile_pool(name="singles", bufs=1))
        mm_ps = ctx.enter_context(tc.tile_pool(name="mm_ps", bufs=2, space="PSUM"))
        qk_ps_pool = ctx.enter_context(tc.tile_pool(name="qk_ps", bufs=3, space="PSUM"))
        av_ps_pool = ctx.enter_context(tc.tile_pool(name="av_ps", bufs=3, space="PSUM"))
        pt_pool = ctx.enter_context(tc.tile_pool(name="pt", bufs=8))
        small = ctx.enter_context(tc.tile_pool(name="small", bufs=4))
        dram = ctx.enter_context(tc.tile_pool(name="dram", bufs=4, space="DRAM"))
        ost = ctx.enter_context(tc.tile_pool(name="ost", bufs=3))

        # ---- resident tensors (split DMAs so compute starts early) ----
        xt_sb = singles.tile([P, n_ci, T], BF16)
        wq_sb = singles.tile([P, n_ci, CH], BF16)
        wk_sb = singles.tile([P, n_ci, CH], BF16)
        wv_sb = singles.tile([P, n_ci, CH], BF16)
        bq_sb = singles.tile([P, n_dt], F32)
        nc.sync.dma_start(bq_sb, bq)
        bk_sb = singles.tile([P, n_dt], F32)
        nc.sync.dma_start(bk_sb, bk)
        vb1_sb = singles.tile([P, CH + HPC], F32)
        nc.sync.dma_start(vb1_sb, vb1)
        mask_sb = singles.tile([P, 4, QC], BF16)
        nc.sync.dma_start(mask_sb, masks.rearrange("r p q -> p r q"))
        for ci in range(n_ci):
            rows = slice(ci * P, (ci + 1) * P)
            nc.sync.dma_start(xt_sb[:, ci, :], xt[rows, :])
            nc.sync.dma_start(wv_sb[:, ci, :], wv[rows, :])
            nc.sync.dma_start(wq_sb[:, ci, :], wq[rows, :])
            nc.sync.dma_start(wk_sb[:, ci, :], wk[rows, :])
        wc_sb = singles.tile([P, n_dt, N_EMBD], BF16)
        for pr in range(n_dt):
            nc.sync.dma_start(wc_sb[:, pr, :], wc[pr * P:(pr + 1) * P, :])

        qt_sb = singles.tile([P, n_dt, T], BF16)   # [d%128, head-pair, t]
        kt_sb = singles.tile([P, n_dt, T], BF16)
        v1_sb = singles.tile([P, n_tt, HD + 1, HPC], BF16)
        yt_sb = singles.tile([P, n_dt, T], BF16)   # attention out (normalized)

        # ones column of v1 (written once, before any V tile is consumed)
        nc.vector.tensor_copy(
            out=v1_sb[:, :, HD, :],
            in_=vb1_sb[:, None, CH:].to_broadcast((P, n_tt, HPC)),
        )

        # ---- work units ----
        def emit_v(tt):
            ps = mm_ps.tile([P, CH], F32, tag="mm")
            for ci in range(n_ci):
                nc.tensor.matmul(
                    ps,
                    lhsT=xt_sb[:, ci, tt * P:(tt + 1) * P],
                    rhs=wv_sb[:, ci, :],
                    start=(ci == 0), stop=(ci == n_ci - 1),
                )
            nc.vector.tensor_add(
                out=v1_sb[:, tt, 0:HD, :],
                in0=ps.rearrange("p (j h) -> p j h", h=HPC),
                in1=vb1_sb[:, 0:CH].rearrange("p (j h) -> p j h", h=HPC),
            )

        def emit_proj(w_sb, b_sb, o_sb, pr, tcn):
            ps = mm_ps.tile([P, QC], F32, tag="mm")
            for ci in range(n_ci):
                nc.tensor.matmul(
                    ps,
                    lhsT=w_sb[:, ci, pr * P:(pr + 1) * P],
                    rhs=xt_sb[:, ci, tcn * QC:(tcn + 1) * QC],
                    start=(ci == 0), stop=(ci == n_ci - 1),
                )
            nc.vector.tensor_add(
                out=o_sb[:, pr, tcn * QC:(tcn + 1) * QC],
                in0=ps,
                in1=b_sb[:, pr, None].to_broadcast((P, QC)),
            )

        def emit_cproj(tt, oc):
            ps = mm_ps.tile([P, QC], F32, tag="mm")
            for pr in range(n_dt):
                nc.tensor.matmul(
                    ps,
                    lhsT=yt_sb[:, pr, tt * P:(tt + 1) * P],
                    rhs=wc_sb[:, pr, oc * QC:(oc + 1) * QC],
                    start=(pr == 0), stop=(pr == n_dt - 1),
                )
            st = ost.tile([P, QC], F32, tag="ost")
            nc.vector.tensor_copy(out=st, in_=ps)
            nc.sync.dma_start(
                out=out[tt * P:(tt + 1) * P, oc * QC:(oc + 1) * QC], in_=st,
            )

        # ---- fused pipeline over q-chunks ----
        # prologue: V for chunk 0 and Q/K projections for (pr=0, chunk 0)
        for tt in range(QC // P):
            emit_v(tt)
        emit_proj(wq_sb, bq_sb, qt_sb, 0, 0)
        emit_proj(wk_sb, bk_sb, kt_sb, 0, 0)

        for qc in range(n_qc):
            q0 = qc * QC
            nkt = (q0 + QC) // P  # causal: k-tiles 0..nkt-1

            # filler units: projections for chunk qc+1, c_proj for chunk qc-1
            fillers = []
            if qc + 1 < n_qc:
                for tt in range((qc + 1) * (QC // P), (qc + 2) * (QC // P)):
                    fillers.append(("v", tt))
                for pr in range(n_dt):
                    fillers.append(("q", pr, qc + 1))
                    fillers.append(("k", pr, qc + 1))
            if qc >= 1:
                for tt in range((qc - 1) * (QC // P), qc * (QC // P)):
                    for oc in range(n_oc):
                        fillers.append(("c", tt, oc))
            n_slots = n_dt * nkt
            per_slot = len(fillers) / n_slots
            facc = 0.0
            fi = 0

            for pr in range(n_dt):
                if qc == 0 and pr >= 1:
                    emit_proj(wq_sb, bq_sb, qt_sb, pr, 0)
                    emit_proj(wk_sb, bk_sb, kt_sb, pr, 0)
                av_A = av_ps_pool.tile([HD + 1, QC], F32, tag="av")
                av_B = av_ps_pool.tile([HD + 1, QC], F32, tag="av")
                pend = None  # delayed P@V: (kt, relq, pt2)
                for kt in range(nkt):
                    k0 = kt * P
                    rel = kt - (q0 // P)  # >=0 on diagonal block tiles
                    relq = rel * P if rel > 0 else 0
                    pts = []
                    for hi, prt in enumerate((slice(0, HD), slice(HD, P))):
                        qk = qk_ps_pool.tile([P, QC], F32, tag="qk")
                        nc.tensor.matmul(
                            qk[:, relq:],
                            lhsT=kt_sb[prt, pr, k0:k0 + P],
                            rhs=qt_sb[prt, pr, q0 + relq:q0 + QC],
                            start=True, stop=True,
                            tile_position=(prt.start, 0) if pack_qk else None,
                        )
                        pt = pt_pool.tile([P, QC], BF16, tag="pt")
                        nc.scalar.activation(
                            pt[:, relq:], qk[:, relq:],
                            mybir.ActivationFunctionType.Exp, scale=SCALE,
                        )
                        if rel >= 0:
                            nc.vector.tensor_mul(
                                pt[:, relq:], pt[:, relq:],
                                mask_sb[:, rel, relq:],
                            )
                        pts.append(pt)
                    if pend is not None:
                        pkt, prelq, ppts = pend
                        for hi, av in enumerate((av_A, av_B)):
                            nc.tensor.matmul(
                                av[:, prelq:],
                                lhsT=v1_sb[:, pkt, :, 2 * pr + hi],
                                rhs=ppts[hi][:, prelq:],
                                start=(pkt == 0), stop=False,
                            )
                    pend = (kt, relq, pts)
                    # interleave filler work to keep TensorE fed
                    facc += per_slot
                    while facc >= 1.0 and fi < len(fillers):
                        f = fillers[fi]
                        fi += 1
                        facc -= 1.0
                        if f[0] == "v":
                            emit_v(f[1])
                        elif f[0] == "q":
                            emit_proj(wq_sb, bq_sb, qt_sb, f[1], f[2])
                        elif f[0] == "k":
                            emit_proj(wk_sb, bk_sb, kt_sb, f[1], f[2])
                        else:
                            emit_cproj(f[1], f[2])
                # final delayed P@V
                pkt, prelq, ppts = pend
                for hi, av in enumerate((av_A, av_B)):
                    nc.tensor.matmul(
                        av[:, prelq:],
                        lhsT=v1_sb[:, pkt, :, 2 * pr + hi],
                        rhs=ppts[hi][:, prelq:],
                        start=(pkt == 0), stop=True,
                    )
                # normalize: y[d, q] = av[d, q] / av[HD, q]
                for hi, av in enumerate((av_A, av_B)):
                    r_sb = small.tile([HD + 1, QC], F32, tag="recip")
                    nc.vector.reciprocal(out=r_sb[HD:HD + 1, :],
                                         in_=av[HD:HD + 1, :])
                    dscr = dram.tile([1, QC], F32, tag="dbounce")
                    nc.gpsimd.dma_start(out=dscr, in_=r_sb[HD:HD + 1, :])
                    den_sb = small.tile([HD, QC], F32, tag="den")
                    nc.gpsimd.dma_start(
                        out=den_sb,
                        in_=bass.AP(tensor=dscr.tensor, offset=dscr.offset,
                                    ap=[[0, HD]] + list(dscr.ap[1:])),
                    )
                    if hi == 0:
                        nc.vector.tensor_mul(
                            out=yt_sb[0:HD, pr, q0:q0 + QC],
                            in0=av[0:HD, :], in1=den_sb,
                        )
                    else:
                        tmp = small.tile([HD, QC], BF16, tag="ytmp")
                        nc.vector.tensor_mul(out=tmp, in0=av[0:HD, :], in1=den_sb)
                        nc.sync.dma_start(out=yt_sb[HD:P, pr, q0:q0 + QC], in_=tmp)

            # any leftover fillers for this stage
            while fi < len(fillers):
                f = fillers[fi]
                fi += 1
                if f[0] == "v":
                    emit_v(f[1])
                elif f[0] == "q":
                    emit_proj(wq_sb, bq_sb, qt_sb, f[1], f[2])
                elif f[0] == "k":
                    emit_proj(wk_sb, bk_sb, kt_sb, f[1], f[2])
                else:
                    emit_cproj(f[1], f[2])

        # epilogue: c_proj for the last chunk
        for tt in range((n_qc - 1) * (QC // P), n_qc * (QC // P)):
            for oc in range(n_oc):
                emit_cproj(tt, oc)

    nc.compile()
    return nc


def make_in_maps(x, Wq, bq, Wk, bk, Wv, bv, T=T_FULL):
    """Host-side sharding + layout prep. Returns per-core input dicts."""
    bf = ml_dtypes.bfloat16
    x = np.asarray(x, dtype=np.float32)
    n_dt = CH // P

    # causal masks for the 4 diagonal-relative offsets
    k_idx = np.arange(P)[:, None]
    q_idx = np.arange(QC)[None, :]
    masks = np.stack(
        [(r * P + k_idx <= q_idx) for r in range(4)]
    ).astype(bf)  # [4, 128, 512]

    # head-interleave permutation for Wv columns: new col j*HPC+h = old h*HD+j
    j = np.arange(HD)[:, None]
    h = np.arange(HPC)[None, :]
    perm = (h * HD + j).reshape(-1)  # new[j*HPC+h] <- old[h*HD+j]

    wqT = np.ascontiguousarray(Wq.T).astype(bf)  # [cin, dout]
    wkT = np.ascontiguousarray(Wk.T).astype(bf)
    wvT = np.ascontiguousarray(Wv.T).astype(bf)

    in_maps = []
    for core in range(N_CORES):
        b = core // 2
        hh = core % 2
        cs = slice(hh * CH, (hh + 1) * CH)
        xtb = np.ascontiguousarray(x[b, :T].T).astype(bf)  # [N_EMBD, T]

        bq_arr = np.asarray(bq[cs], np.float32).reshape(n_dt, P).T.copy()
        bk_arr = np.asarray(bk[cs], np.float32).reshape(n_dt, P).T.copy()
        bv_half = np.asarray(bv[cs], np.float32)
        vb1 = np.concatenate([bv_half[perm], np.ones(HPC, np.float32)])
        vb1 = np.broadcast_to(vb1, (P, CH + HPC)).copy()

        in_maps.append({
            "xt": xtb,
            "wq": np.ascontiguousarray(wqT[:, cs]),
            "wk": np.ascontiguousarray(wkT[:, cs]),
            "wv": np.ascontiguousarray(wvT[:, cs][:, perm]),
            "wc": None,  # filled by caller (needs Wc)
            "bq": bq_arr,
            "bk": bk_arr,
            "vb1": vb1,
            "masks": masks,
        })
    return in_maps


_NC_CACHE = {}


def kernel(x, Wq, bq, Wk, bk, Wv, bv, Wc, bc):
    x = np.asarray(x, dtype=np.float32)
    T = x.shape[1]
    key = T
    if key not in _NC_CACHE:
        _NC_CACHE[key] = build_nc(T=T)
    nc = _NC_CACHE[key]

    in_maps = make_in_maps(x, Wq, bq, Wk, bk, Wv, bv, T=T)
    wcT = np.ascontiguousarray(np.asarray(Wc, np.float32).T).astype(
        ml_dtypes.bfloat16)  # [cin, cout]
    for core in range(N_CORES):
        hh = core % 2
        in_maps[core]["wc"] = np.ascontiguousarray(wcT[hh * CH:(hh + 1) * CH, :])

    res = run_bass_kernel_spmd(nc, in_maps, core_ids=list(range(N_CORES)))

    bc = np.asarray(bc, np.float32)
    out = np.empty((B, T, N_EMBD), np.float32)
    for b in range(B):
        out[b] = res.results[2 * b]["out"] + res.results[2 * b + 1]["out"] + bc
    return out


# revision 23
# speedup vs baseline: 1.3103x; 1.0465x over previous
"""Causal self-attention kernel for 8 TRN2 NeuronCores.

Sharding: 8 cores = 4 batches x 2 head-groups (8 heads / 512 channels each).
Each core computes q/k/v projections for its head half, causal attention for
its 8 heads, and a partial c_proj contracted over its 512 channels. The host
sums the two partials per batch and adds the c_proj bias.

All matmuls run in bf16 with fp32 PSUM accumulation. Host pre-transposes
x and the weight matrices so the device only ever does natural-layout DMAs.

Device layouts (per core):
  xt_sb [128, 8, T]        x^T tiles: [cin%128, cin//128, t]
  qt/kt_sb [128, 4, T]     Q^T/K^T: [d%128, head-pair, t] (head 2p: rows 0-63)
  v1_sb [128, T//128, 65, 8]  V interleaved [t%128, t//128, j, h]; col j<64 is
                           V_h[d=j] (host permutes Wv cols to dd*8+h), j=64
                           is the ones column used to accumulate the softmax
                           denominator inside the P@V matmul (M=65).

Scores are computed transposed (S^T[k, q]) so softmax'd probabilities feed
P@V directly as the moving operand; exp runs on ScalarE from PSUM with the
1/sqrt(hd) scale folded in; diagonal blocks are masked post-exp and trimmed
to their valid q-range. The attention inner loop is ScalarE-paced, so
projection / c_proj matmul groups are interleaved into it at k-tile
granularity to keep TensorE busy, and P@V is emitted one k-tile behind the
exp that feeds it.
"""

import numpy as np
import ml_dtypes
from contextlib import ExitStack

import concourse.bass as bass
import concourse.tile as tile
from concourse import bacc, mybir
from concourse.bass_utils import run_bass_kernel_spmd

BF16 = mybir.dt.bfloat16
F32 = mybir.dt.float32

N_EMBD = 1024
N_HEAD = 16
B = 4
T_FULL = 2048
HD = 64           # head dim
HPC = 8           # heads per core
CH = HPC * HD     # channels per core = 512
N_CORES = 8
SCALE = 1.0 / 8.0  # 1/sqrt(HD)

P = 128           # partitions
QC = 512          # q-chunk (matmul free dim)


def build_nc(T=T_FULL, pack_qk=True):
    """Build the per-core Bass module (same program on every core)."""
    n_tt = T // P          # 128-row tiles along T
    n_qc = T // QC         # 512-wide chunks along T
    n_ci = N_EMBD // P     # contraction tiles over the full embed dim
    n_dt = CH // P         # d-tiles of this core's 512 channels (= head pairs)
    n_oc = N_EMBD // QC

    nc = bacc.Bacc("TRN2", target_bir_lowering=False, debug=False)

    xt = nc.dram_tensor("xt", [N_EMBD, T], BF16, kind="ExternalInput").ap()
    wqkv = nc.dram_tensor("wqkv", [N_EMBD, 3 * CH], BF16,
                          kind="ExternalInput").ap()
    wc = nc.dram_tensor("wc", [CH, N_EMBD], BF16, kind="ExternalInput").ap()
    bq = nc.dram_tensor("bq", [P, n_dt], F32, kind="ExternalInput").ap()
    bk = nc.dram_tensor("bk", [P, n_dt], F32, kind="ExternalInput").ap()
    vb1 = nc.dram_tensor("vb1", [P, CH + HPC], F32, kind="ExternalInput").ap()
    masks = nc.dram_tensor("masks", [P, 4 * QC], BF16, kind="ExternalInput").ap()
    out = nc.dram_tensor("out", [T, N_EMBD], F32, kind="ExternalOutput").ap()

    with tile.TileContext(nc) as tc, ExitStack() as ctx:
        singles = ctx.enter_context(tc.tile_pool(name="singles", bufs=1))
        mm_ps = ctx.enter_context(tc.tile_pool(name="mm_ps", bufs=2, space="PSUM"))
        qk_ps_pool = ctx.enter_context(tc.tile_pool(name="qk_ps", bufs=3, space="PSUM"))
        av_ps_pool = ctx.enter_context(tc.tile_pool(name="av_ps", bufs=3, space="PSUM"))
        pt_pool = ctx.enter_context(tc.tile_pool(name="pt", bufs=8))
        small = ctx.enter_context(tc.tile_pool(name="small", bufs=4))
        dram = ctx.enter_context(tc.tile_pool(name="dram", bufs=4, space="DRAM"))
        ost = ctx.enter_context(tc.tile_pool(name="ost", bufs=3))

        # ---- resident tensors (split DMAs so compute starts early) ----
        xt_sb = singles.tile([P, n_ci, T], BF16)
        wqkv_sb = singles.tile([P, n_ci, 3 * CH], BF16)
        wq_sb = wqkv_sb[:, :, 0:CH]
        wk_sb = wqkv_sb[:, :, CH:2 * CH]
        wv_sb = wqkv_sb[:, :, 2 * CH:3 * CH]
        bq_sb = singles.tile([P, n_dt], F32)
        nc.sync.dma_start(bq_sb, bq)
        bk_sb = singles.tile([P, n_dt], F32)
        nc.sync.dma_start(bk_sb, bk)
        vb1_sb = singles.tile([P, CH + HPC], F32)
        nc.sync.dma_start(vb1_sb, vb1)
        mask_sb = singles.tile([P, 4, QC], BF16)
        nc.sync.dma_start(mask_sb, masks.rearrange("p (r q) -> p r q", r=4))
        for ci in range(n_ci):
            rows = slice(ci * P, (ci + 1) * P)
            nc.sync.dma_start(xt_sb[:, ci, :], xt[rows, :])
            nc.sync.dma_start(wqkv_sb[:, ci, :], wqkv[rows, :])
        wc_sb = singles.tile([P, n_dt, N_EMBD], BF16)
        for pr in range(n_dt):
            nc.sync.dma_start(wc_sb[:, pr, :], wc[pr * P:(pr + 1) * P, :])

        qt_sb = singles.tile([P, n_dt, T], BF16)   # [d%128, head-pair, t]
        kt_sb = singles.tile([P, n_dt, T], BF16)
        v1_sb = singles.tile([P, n_tt, HD + 1, HPC], BF16)
        yt_sb = singles.tile([P, n_dt, T], BF16)   # attention out (normalized)

        # ones column of v1 (written once, before any V tile is consumed)
        nc.vector.tensor_copy(
            out=v1_sb[:, :, HD, :],
            in_=vb1_sb[:, None, CH:].to_broadcast((P, n_tt, HPC)),
        )

        # ---- work units ----
        def emit_v(tt):
            ps = mm_ps.tile([P, CH], F32, tag="mm")
            for ci in range(n_ci):
                nc.tensor.matmul(
                    ps,
                    lhsT=xt_sb[:, ci, tt * P:(tt + 1) * P],
                    rhs=wv_sb[:, ci, :],
                    start=(ci == 0), stop=(ci == n_ci - 1),
                )
            nc.vector.tensor_add(
                out=v1_sb[:, tt, 0:HD, :],
                in0=ps.rearrange("p (j h) -> p j h", h=HPC),
                in1=vb1_sb[:, 0:CH].rearrange("p (j h) -> p j h", h=HPC),
            )

        def emit_proj(w_sb, b_sb, o_sb, pr, tcn):
            ps = mm_ps.tile([P, QC], F32, tag="mm")
            for ci in range(n_ci):
                nc.tensor.matmul(
                    ps,
                    lhsT=w_sb[:, ci, pr * P:(pr + 1) * P],
                    rhs=xt_sb[:, ci, tcn * QC:(tcn + 1) * QC],
                    start=(ci == 0), stop=(ci == n_ci - 1),
                )
            nc.vector.tensor_add(
                out=o_sb[:, pr, tcn * QC:(tcn + 1) * QC],
                in0=ps,
                in1=b_sb[:, pr, None].to_broadcast((P, QC)),
            )

        def emit_cproj(tt, oc, on_act=False):
            ps = mm_ps.tile([P, QC], F32, tag="mm")
            for pr in range(n_dt):
                nc.tensor.matmul(
                    ps,
                    lhsT=yt_sb[:, pr, tt * P:(tt + 1) * P],
                    rhs=wc_sb[:, pr, oc * QC:(oc + 1) * QC],
                    start=(pr == 0), stop=(pr == n_dt - 1),
                )
            st = ost.tile([P, QC], F32, tag="ost")
            if on_act:  # ScalarE is idle once attention's exp stream drains
                nc.scalar.activation(st, ps, mybir.ActivationFunctionType.Copy)
            else:
                nc.vector.tensor_copy(out=st, in_=ps)
            nc.sync.dma_start(
                out=out[tt * P:(tt + 1) * P, oc * QC:(oc + 1) * QC], in_=st,
            )

        # ---- fused pipeline over q-chunks ----
        # prologue: V for chunk 0 and Q/K projections for (pr=0, chunk 0)
        for tt in range(QC // P):
            emit_v(tt)
        emit_proj(wq_sb, bq_sb, qt_sb, 0, 0)
        emit_proj(wk_sb, bk_sb, kt_sb, 0, 0)

        for qc in range(n_qc):
            q0 = qc * QC
            nkt = (q0 + QC) // P  # causal: k-tiles 0..nkt-1

            # filler units: projections for chunk qc+1; all deferrable c_proj
            # is pushed into the last chunk, whose attention is ScalarE-bound
            # and leaves TensorE the most idle.
            fillers = []
            if qc + 1 < n_qc:
                for tt in range((qc + 1) * (QC // P), (qc + 2) * (QC // P)):
                    fillers.append(("v", tt))
                for pr in range(n_dt):
                    fillers.append(("q", pr, qc + 1))
                    fillers.append(("k", pr, qc + 1))
            if qc == n_qc - 1:
                for tt in range(0, (n_qc - 1) * (QC // P)):
                    for oc in range(n_oc):
                        fillers.append(("c", tt, oc))
            n_slots = n_dt * nkt
            per_slot = len(fillers) / n_slots
            if qc == n_qc - 1:
                per_slot *= 0.7  # hold filler back for the drain at chunk end
            facc = 0.0
            fi = 0

            for pr in range(n_dt):
                if qc == 0 and pr >= 1:
                    emit_proj(wq_sb, bq_sb, qt_sb, pr, 0)
                    emit_proj(wk_sb, bk_sb, kt_sb, pr, 0)
                av_A = av_ps_pool.tile([HD + 1, QC], F32, tag="av")
                av_B = av_ps_pool.tile([HD + 1, QC], F32, tag="av")
                pend = None  # delayed P@V: (kt, relq, pt2)
                for kt in range(nkt):
                    k0 = kt * P
                    rel = kt - (q0 // P)  # >=0 on diagonal block tiles
                    relq = rel * P if rel > 0 else 0
                    pts = []
                    for hi, prt in enumerate((slice(0, HD), slice(HD, P))):
                        qk = qk_ps_pool.tile([P, QC], F32, tag="qk")
                        nc.tensor.matmul(
                            qk[:, relq:],
                            lhsT=kt_sb[prt, pr, k0:k0 + P],
                            rhs=qt_sb[prt, pr, q0 + relq:q0 + QC],
                            start=True, stop=True,
                            tile_position=(prt.start, 0) if pack_qk else None,
                        )
                        pt = pt_pool.tile([P, QC], BF16, tag="pt")
                        nc.scalar.activation(
                            pt[:, relq:], qk[:, relq:],
                            mybir.ActivationFunctionType.Exp, scale=SCALE,
                        )
                        if rel >= 0:
                            nc.vector.tensor_mul(
                                pt[:, relq:], pt[:, relq:],
                                mask_sb[:, rel, relq:],
                            )
                        pts.append(pt)
                    if pend is not None:
                        pkt, prelq, ppts = pend
                        for hi, av in enumerate((av_A, av_B)):
                            nc.tensor.matmul(
                                av[:, prelq:],
                                lhsT=v1_sb[:, pkt, :, 2 * pr + hi],
                                rhs=ppts[hi][:, prelq:],
                                start=(pkt == 0), stop=False,
                            )
                    pend = (kt, relq, pts)
                    # interleave filler work to keep TensorE fed
                    facc += per_slot
                    while facc >= 1.0 and fi < len(fillers):
                        f = fillers[fi]
                        fi += 1
                        facc -= 1.0
                        if f[0] == "v":
                            emit_v(f[1])
                        elif f[0] == "q":
                            emit_proj(wq_sb, bq_sb, qt_sb, f[1], f[2])
                        elif f[0] == "k":
                            emit_proj(wk_sb, bk_sb, kt_sb, f[1], f[2])
                        else:
                            emit_cproj(f[1], f[2])
                # final delayed P@V
                pkt, prelq, ppts = pend
                for hi, av in enumerate((av_A, av_B)):
                    nc.tensor.matmul(
                        av[:, prelq:],
                        lhsT=v1_sb[:, pkt, :, 2 * pr + hi],
                        rhs=ppts[hi][:, prelq:],
                        start=(pkt == 0), stop=True,
                    )
                # normalize: y[d, q] = av[d, q] / av[HD, q]
                for hi, av in enumerate((av_A, av_B)):
                    r_sb = small.tile([HD + 1, QC], F32, tag="recip")
                    nc.vector.reciprocal(out=r_sb[HD:HD + 1, :],
                                         in_=av[HD:HD + 1, :])
                    dscr = dram.tile([1, QC], F32, tag="dbounce")
                    nc.gpsimd.dma_start(out=dscr, in_=r_sb[HD:HD + 1, :])
                    den_sb = small.tile([HD, QC], F32, tag="den")
                    nc.gpsimd.dma_start(
                        out=den_sb,
                        in_=bass.AP(tensor=dscr.tensor, offset=dscr.offset,
                                    ap=[[0, HD]] + list(dscr.ap[1:])),
                    )
                    if hi == 0:
                        nc.vector.tensor_mul(
                            out=yt_sb[0:HD, pr, q0:q0 + QC],
                            in0=av[0:HD, :], in1=den_sb,
                        )
                    else:
                        tmp = small.tile([HD, QC], BF16, tag="ytmp")
                        nc.vector.tensor_mul(out=tmp, in0=av[0:HD, :], in1=den_sb)
                        nc.sync.dma_start(out=yt_sb[HD:P, pr, q0:q0 + QC], in_=tmp)

                if qc == n_qc - 1:
                    # drain held-back filler while the next pair's exps queue
                    for _ in range(2):
                        if fi < len(fillers):
                            f = fillers[fi]
                            fi += 1
                            if f[0] == "c":
                                emit_cproj(f[1], f[2])

            # any leftover fillers for this stage
            while fi < len(fillers):
                f = fillers[fi]
                fi += 1
                if f[0] == "v":
                    emit_v(f[1])
                elif f[0] == "q":
                    emit_proj(wq_sb, bq_sb, qt_sb, f[1], f[2])
                elif f[0] == "k":
                    emit_proj(wk_sb, bk_sb, kt_sb, f[1], f[2])
                else:
                    emit_cproj(f[1], f[2])

        # epilogue: c_proj for the last chunk (stage copies on idle ScalarE)
        for tt in range((n_qc - 1) * (QC // P), n_qc * (QC // P)):
            for oc in range(n_oc):
                emit_cproj(tt, oc, on_act=True)

    nc.compile()
    return nc


def make_in_maps(x, Wq, bq, Wk, bk, Wv, bv, T=T_FULL):
    """Host-side sharding + layout prep. Returns per-core input dicts."""
    bf = ml_dtypes.bfloat16
    x = np.asarray(x, dtype=np.float32)
    n_dt = CH // P

    # causal masks for the 4 diagonal-relative offsets
    k_idx = np.arange(P)[:, None]
    q_idx = np.arange(QC)[None, :]
    masks = np.concatenate(
        [(r * P + k_idx <= q_idx) for r in range(4)], axis=1
    ).astype(bf)  # [128, 4*512] packed along the free dim

    # head-interleave permutation for Wv columns: new col j*HPC+h = old h*HD+j
    j = np.arange(HD)[:, None]
    h = np.arange(HPC)[None, :]
    perm = (h * HD + j).reshape(-1)  # new[j*HPC+h] <- old[h*HD+j]

    wqT = np.ascontiguousarray(Wq.T).astype(bf)  # [cin, dout]
    wkT = np.ascontiguousarray(Wk.T).astype(bf)
    wvT = np.ascontiguousarray(Wv.T).astype(bf)

    in_maps = []
    for core in range(N_CORES):
        b = core // 2
        hh = core % 2
        cs = slice(hh * CH, (hh + 1) * CH)
        xtb = np.ascontiguousarray(x[b, :T].T).astype(bf)  # [N_EMBD, T]

        bq_arr = np.asarray(bq[cs], np.float32).reshape(n_dt, P).T.copy()
        bk_arr = np.asarray(bk[cs], np.float32).reshape(n_dt, P).T.copy()
        bv_half = np.asarray(bv[cs], np.float32)
        vb1 = np.concatenate([bv_half[perm], np.ones(HPC, np.float32)])
        vb1 = np.broadcast_to(vb1, (P, CH + HPC)).copy()

        in_maps.append({
            "xt": xtb,
            "wqkv": np.ascontiguousarray(np.concatenate(
                [wqT[:, cs], wkT[:, cs], wvT[:, cs][:, perm]], axis=1)),
            "wc": None,  # filled by caller (needs Wc)
            "bq": bq_arr,
            "bk": bk_arr,
            "vb1": vb1,
            "masks": masks,
        })
    return in_maps


_NC_CACHE = {}


def kernel(x, Wq, bq, Wk, bk, Wv, bv, Wc, bc):
    x = np.asarray(x, dtype=np.float32)
    T = x.shape[1]
    key = T
    if key not in _NC_CACHE:
        _NC_CACHE[key] = build_nc(T=T)
    nc = _NC_CACHE[key]

    in_maps = make_in_maps(x, Wq, bq, Wk, bk, Wv, bv, T=T)
    wcT = np.ascontiguousarray(np.asarray(Wc, np.float32).T).astype(
        ml_dtypes.bfloat16)  # [cin, cout]
    for core in range(N_CORES):
        hh = core % 2
        in_maps[core]["wc"] = np.ascontiguousarray(wcT[hh * CH:(hh + 1) * CH, :])

    res = run_bass_kernel_spmd(nc, in_maps, core_ids=list(range(N_CORES)))

    bc = np.asarray(bc, np.float32)
    out = np.empty((B, T, N_EMBD), np.float32)
    for b in range(B):
        out[b] = res.results[2 * b]["out"] + res.results[2 * b + 1]["out"] + bc
    return out


# revision 28
# speedup vs baseline: 1.3164x; 1.0047x over previous
"""Causal self-attention kernel for 8 TRN2 NeuronCores.

Sharding: 8 cores = 4 batches x 2 head-groups (8 heads / 512 channels each).
Each core computes q/k/v projections for its head half, causal attention for
its 8 heads, and a partial c_proj contracted over its 512 channels. The host
sums the two partials per batch and adds the c_proj bias.

All matmuls run in bf16 with fp32 PSUM accumulation. Host pre-transposes
x and the weight matrices so the device only ever does natural-layout DMAs.

Device layouts (per core):
  xt_sb [128, 8, T]        x^T tiles: [cin%128, cin//128, t]
  qt/kt_sb [128, 4, T]     Q^T/K^T: [d%128, head-pair, t] (head 2p: rows 0-63)
  v1_sb [128, T//128, 65, 8]  V interleaved [t%128, t//128, j, h]; col j<64 is
                           V_h[d=j] (host permutes Wv cols to dd*8+h), j=64
                           is the ones column used to accumulate the softmax
                           denominator inside the P@V matmul (M=65).

Scores are computed transposed (S^T[k, q]) so softmax'd probabilities feed
P@V directly as the moving operand; exp runs on ScalarE from PSUM with the
1/sqrt(hd) scale folded in; diagonal blocks are masked post-exp and trimmed
to their valid q-range. The attention inner loop is ScalarE-paced, so
projection / c_proj matmul groups are interleaved into it at k-tile
granularity to keep TensorE busy, and P@V is emitted one k-tile behind the
exp that feeds it.
"""

import numpy as np
import ml_dtypes
from contextlib import ExitStack

import concourse.bass as bass
import concourse.tile as tile
from concourse import bacc, mybir
from concourse.bass_utils import run_bass_kernel_spmd

BF16 = mybir.dt.bfloat16
F32 = mybir.dt.float32

N_EMBD = 1024
N_HEAD = 16
B = 4
T_FULL = 2048
HD = 64           # head dim
HPC = 8           # heads per core
CH = HPC * HD     # channels per core = 512
N_CORES = 8
SCALE = 1.0 / 8.0  # 1/sqrt(HD)

P = 128           # partitions
QC = 512          # q-chunk (matmul free dim)


def build_nc(T=T_FULL, pack_qk=True):
    """Build the per-core Bass module (same program on every core)."""
    n_tt = T // P          # 128-row tiles along T
    n_qc = T // QC         # 512-wide chunks along T
    n_ci = N_EMBD // P     # contraction tiles over the full embed dim
    n_dt = CH // P         # d-tiles of this core's 512 channels (= head pairs)
    n_oc = N_EMBD // QC

    nc = bacc.Bacc("TRN2", target_bir_lowering=False, debug=False)

    xt = nc.dram_tensor("xt", [N_EMBD, T], BF16, kind="ExternalInput").ap()
    wqkv = nc.dram_tensor("wqkv", [N_EMBD, 3 * CH], BF16,
                          kind="ExternalInput").ap()
    wc = nc.dram_tensor("wc", [CH, N_EMBD], BF16, kind="ExternalInput").ap()
    bq = nc.dram_tensor("bq", [P, n_dt], F32, kind="ExternalInput").ap()
    bk = nc.dram_tensor("bk", [P, n_dt], F32, kind="ExternalInput").ap()
    vb1 = nc.dram_tensor("vb1", [P, CH + HPC], F32, kind="ExternalInput").ap()
    masks = nc.dram_tensor("masks", [P, 4 * QC], BF16, kind="ExternalInput").ap()
    out = nc.dram_tensor("out", [T, N_EMBD], F32, kind="ExternalOutput").ap()

    with tile.TileContext(nc) as tc, ExitStack() as ctx:
        singles = ctx.enter_context(tc.tile_pool(name="singles", bufs=1))
        mm_ps = ctx.enter_context(tc.tile_pool(name="mm_ps", bufs=2, space="PSUM"))
        qk_ps_pool = ctx.enter_context(tc.tile_pool(name="qk_ps", bufs=3, space="PSUM"))
        av_ps_pool = ctx.enter_context(tc.tile_pool(name="av_ps", bufs=3, space="PSUM"))
        pt_pool = ctx.enter_context(tc.tile_pool(name="pt", bufs=12))
        small = ctx.enter_context(tc.tile_pool(name="small", bufs=4))
        dram = ctx.enter_context(tc.tile_pool(name="dram", bufs=4, space="DRAM"))
        ost = ctx.enter_context(tc.tile_pool(name="ost", bufs=3))

        # ---- resident tensors (split DMAs so compute starts early) ----
        xt_sb = singles.tile([P, n_ci, T], BF16)
        wqkv_sb = singles.tile([P, n_ci, 3 * CH], BF16)
        wq_sb = wqkv_sb[:, :, 0:CH]
        wk_sb = wqkv_sb[:, :, CH:2 * CH]
        wv_sb = wqkv_sb[:, :, 2 * CH:3 * CH]
        bq_sb = singles.tile([P, n_dt], F32)
        nc.sync.dma_start(bq_sb, bq)
        bk_sb = singles.tile([P, n_dt], F32)
        nc.sync.dma_start(bk_sb, bk)
        vb1_sb = singles.tile([P, CH + HPC], F32)
        nc.sync.dma_start(vb1_sb, vb1)
        mask_sb = singles.tile([P, 4, QC], BF16)
        nc.sync.dma_start(mask_sb, masks.rearrange("p (r q) -> p r q", r=4))
        for ci in range(n_ci):
            rows = slice(ci * P, (ci + 1) * P)
            nc.sync.dma_start(xt_sb[:, ci, :], xt[rows, :])
            nc.sync.dma_start(wqkv_sb[:, ci, :], wqkv[rows, :])
        wc_sb = singles.tile([P, n_dt, N_EMBD], BF16)
        for pr in range(n_dt):
            nc.sync.dma_start(wc_sb[:, pr, :], wc[pr * P:(pr + 1) * P, :])

        qt_sb = singles.tile([P, n_dt, T], BF16)   # [d%128, head-pair, t]
        kt_sb = singles.tile([P, n_dt, T], BF16)
        v1_sb = singles.tile([P, n_tt, HD + 1, HPC], BF16)
        yt_sb = singles.tile([P, n_dt, T], BF16)   # attention out (normalized)

        # ones column of v1 (written once, before any V tile is consumed)
        nc.vector.tensor_copy(
            out=v1_sb[:, :, HD, :],
            in_=vb1_sb[:, None, CH:].to_broadcast((P, n_tt, HPC)),
        )

        # ---- work units ----
        def emit_v(tt):
            ps = mm_ps.tile([P, CH], F32, tag="mm")
            for ci in range(n_ci):
                nc.tensor.matmul(
                    ps,
                    lhsT=xt_sb[:, ci, tt * P:(tt + 1) * P],
                    rhs=wv_sb[:, ci, :],
                    start=(ci == 0), stop=(ci == n_ci - 1),
                )
            nc.vector.tensor_add(
                out=v1_sb[:, tt, 0:HD, :],
                in0=ps.rearrange("p (j h) -> p j h", h=HPC),
                in1=vb1_sb[:, 0:CH].rearrange("p (j h) -> p j h", h=HPC),
            )

        def emit_proj(w_sb, b_sb, o_sb, pr, tcn):
            ps = mm_ps.tile([P, QC], F32, tag="mm")
            for ci in range(n_ci):
                nc.tensor.matmul(
                    ps,
                    lhsT=w_sb[:, ci, pr * P:(pr + 1) * P],
                    rhs=xt_sb[:, ci, tcn * QC:(tcn + 1) * QC],
                    start=(ci == 0), stop=(ci == n_ci - 1),
                )
            nc.vector.tensor_add(
                out=o_sb[:, pr, tcn * QC:(tcn + 1) * QC],
                in0=ps,
                in1=b_sb[:, pr, None].to_broadcast((P, QC)),
            )

        def emit_cproj(tt, oc, on_act=False):
            ps = mm_ps.tile([P, QC], F32, tag="mm")
            for pr in range(n_dt):
                nc.tensor.matmul(
                    ps,
                    lhsT=yt_sb[:, pr, tt * P:(tt + 1) * P],
                    rhs=wc_sb[:, pr, oc * QC:(oc + 1) * QC],
                    start=(pr == 0), stop=(pr == n_dt - 1),
                )
            st = ost.tile([P, QC], F32, tag="ost")
            if on_act:  # ScalarE is idle once attention's exp stream drains
                nc.scalar.activation(st, ps, mybir.ActivationFunctionType.Copy)
            else:
                nc.vector.tensor_copy(out=st, in_=ps)
            nc.sync.dma_start(
                out=out[tt * P:(tt + 1) * P, oc * QC:(oc + 1) * QC], in_=st,
            )

        # ---- fused pipeline over q-chunks ----
        # prologue: V for chunk 0 and Q/K projections for (pr=0, chunk 0)
        for tt in range(QC // P):
            emit_v(tt)
        emit_proj(wq_sb, bq_sb, qt_sb, 0, 0)
        emit_proj(wk_sb, bk_sb, kt_sb, 0, 0)

        for qc in range(n_qc):
            q0 = qc * QC
            nkt = (q0 + QC) // P  # causal: k-tiles 0..nkt-1

            # filler units: projections for chunk qc+1; all deferrable c_proj
            # is pushed into the last chunk, whose attention is ScalarE-bound
            # and leaves TensorE the most idle.
            fillers = []
            if qc + 1 < n_qc:
                for tt in range((qc + 1) * (QC // P), (qc + 2) * (QC // P)):
                    fillers.append(("v", tt))
                for pr in range(n_dt):
                    fillers.append(("q", pr, qc + 1))
                    fillers.append(("k", pr, qc + 1))
            if qc == n_qc - 1:
                for tt in range(0, (n_qc - 1) * (QC // P)):
                    for oc in range(n_oc):
                        fillers.append(("c", tt, oc))
            n_slots = n_dt * nkt
            per_slot = len(fillers) / n_slots
            if qc == n_qc - 1:
                per_slot *= 0.7  # hold filler back for the drain at chunk end
            facc = 0.0
            fi = 0

            for pr in range(n_dt):
                if qc == 0 and pr >= 1:
                    emit_proj(wq_sb, bq_sb, qt_sb, pr, 0)
                    emit_proj(wk_sb, bk_sb, kt_sb, pr, 0)
                av_A = av_ps_pool.tile([HD + 1, QC], F32, tag="av")
                av_B = av_ps_pool.tile([HD + 1, QC], F32, tag="av")
                pend = None  # delayed P@V: (kt, relq, pt2)
                for kt in range(nkt):
                    k0 = kt * P
                    rel = kt - (q0 // P)  # >=0 on diagonal block tiles
                    relq = rel * P if rel > 0 else 0
                    pts = []
                    for hi, prt in enumerate((slice(0, HD), slice(HD, P))):
                        qk = qk_ps_pool.tile([P, QC], F32, tag="qk")
                        nc.tensor.matmul(
                            qk[:, relq:],
                            lhsT=kt_sb[prt, pr, k0:k0 + P],
                            rhs=qt_sb[prt, pr, q0 + relq:q0 + QC],
                            start=True, stop=True,
                            tile_position=(prt.start, 0) if pack_qk else None,
                        )
                        pt = pt_pool.tile([P, QC], BF16, tag="pt")
                        nc.scalar.activation(
                            pt[:, relq:], qk[:, relq:],
                            mybir.ActivationFunctionType.Exp, scale=SCALE,
                        )
                        if rel >= 0:
                            nc.vector.tensor_mul(
                                pt[:, relq:], pt[:, relq:],
                                mask_sb[:, rel, relq:],
                            )
                        pts.append(pt)
                    if pend is not None:
                        pkt, prelq, ppts = pend
                        for hi, av in enumerate((av_A, av_B)):
                            nc.tensor.matmul(
                                av[:, prelq:],
                                lhsT=v1_sb[:, pkt, :, 2 * pr + hi],
                                rhs=ppts[hi][:, prelq:],
                                start=(pkt == 0), stop=False,
                            )
                    pend = (kt, relq, pts)
                    # interleave filler work to keep TensorE fed
                    facc += per_slot
                    while facc >= 1.0 and fi < len(fillers):
                        f = fillers[fi]
                        fi += 1
                        facc -= 1.0
                        if f[0] == "v":
                            emit_v(f[1])
                        elif f[0] == "q":
                            emit_proj(wq_sb, bq_sb, qt_sb, f[1], f[2])
                        elif f[0] == "k":
                            emit_proj(wk_sb, bk_sb, kt_sb, f[1], f[2])
                        else:
                            emit_cproj(f[1], f[2])
                # final delayed P@V
                pkt, prelq, ppts = pend
                for hi, av in enumerate((av_A, av_B)):
                    nc.tensor.matmul(
                        av[:, prelq:],
                        lhsT=v1_sb[:, pkt, :, 2 * pr + hi],
                        rhs=ppts[hi][:, prelq:],
                        start=(pkt == 0), stop=True,
                    )
                # normalize: y[d, q] = av[d, q] / av[HD, q]; one DRAM bounce
                # broadcasts both heads' fp32 reciprocal rows across partitions
                r_sb = small.tile([HD + 1, 2, QC], F32, tag="recip")
                nc.vector.reciprocal(out=r_sb[HD:HD + 1, 0, :],
                                     in_=av_A[HD:HD + 1, :])
                nc.vector.reciprocal(out=r_sb[HD:HD + 1, 1, :],
                                     in_=av_B[HD:HD + 1, :])
                dscr = dram.tile([1, 2 * QC], F32, tag="dbounce")
                nc.gpsimd.dma_start(
                    out=dscr, in_=r_sb[HD:HD + 1].rearrange("p a q -> p (a q)"))
                den_sb = small.tile([HD, 2, QC], F32, tag="den")
                nc.gpsimd.dma_start(
                    out=den_sb,
                    in_=bass.AP(tensor=dscr.tensor, offset=dscr.offset,
                                ap=[[0, HD], [QC, 2], [1, QC]]),
                )
                nc.vector.tensor_mul(
                    out=yt_sb[0:HD, pr, q0:q0 + QC],
                    in0=av_A[0:HD, :], in1=den_sb[:, 0, :],
                )
                tmp = small.tile([HD, QC], BF16, tag="ytmp")
                nc.vector.tensor_mul(out=tmp, in0=av_B[0:HD, :],
                                     in1=den_sb[:, 1, :])
                nc.sync.dma_start(out=yt_sb[HD:P, pr, q0:q0 + QC], in_=tmp)

                if qc == n_qc - 1:
                    # drain held-back filler while the next pair's exps queue
                    for _ in range(2):
                        if fi < len(fillers):
                            f = fillers[fi]
                            fi += 1
                            if f[0] == "c":
                                emit_cproj(f[1], f[2])

            # any leftover fillers for this stage
            while fi < len(fillers):
                f = fillers[fi]
                fi += 1
                if f[0] == "v":
                    emit_v(f[1])
                elif f[0] == "q":
                    emit_proj(wq_sb, bq_sb, qt_sb, f[1], f[2])
                elif f[0] == "k":
                    emit_proj(wk_sb, bk_sb, kt_sb, f[1], f[2])
                else:
                    emit_cproj(f[1], f[2])

        # epilogue: c_proj for the last chunk (stage copies on idle ScalarE)
        for tt in range((n_qc - 1) * (QC // P), n_qc * (QC // P)):
            for oc in range(n_oc):
                emit_cproj(tt, oc, on_act=True)

    nc.compile()
    return nc


def make_in_maps(x, Wq, bq, Wk, bk, Wv, bv, T=T_FULL):
    """Host-side sharding + layout prep. Returns per-core input dicts."""
    bf = ml_dtypes.bfloat16
    x = np.asarray(x, dtype=np.float32)
    n_dt = CH // P

    # causal masks for the 4 diagonal-relative offsets
    k_idx = np.arange(P)[:, None]
    q_idx = np.arange(QC)[None, :]
    masks = np.concatenate(
        [(r * P + k_idx <= q_idx) for r in range(4)], axis=1
    ).astype(bf)  # [128, 4*512] packed along the free dim

    # head-interleave permutation for Wv columns: new col j*HPC+h = old h*HD+j
    j = np.arange(HD)[:, None]
    h = np.arange(HPC)[None, :]
    perm = (h * HD + j).reshape(-1)  # new[j*HPC+h] <- old[h*HD+j]

    wqT = np.ascontiguousarray(Wq.T).astype(bf)  # [cin, dout]
    wkT = np.ascontiguousarray(Wk.T).astype(bf)
    wvT = np.ascontiguousarray(Wv.T).astype(bf)

    in_maps = []
    for core in range(N_CORES):
        b = core // 2
        hh = core % 2
        cs = slice(hh * CH, (hh + 1) * CH)
        xtb = np.ascontiguousarray(x[b, :T].T).astype(bf)  # [N_EMBD, T]

        bq_arr = np.asarray(bq[cs], np.float32).reshape(n_dt, P).T.copy()
        bk_arr = np.asarray(bk[cs], np.float32).reshape(n_dt, P).T.copy()
        bv_half = np.asarray(bv[cs], np.float32)
        vb1 = np.concatenate([bv_half[perm], np.ones(HPC, np.float32)])
        vb1 = np.broadcast_to(vb1, (P, CH + HPC)).copy()

        in_maps.append({
            "xt": xtb,
            "wqkv": np.ascontiguousarray(np.concatenate(
                [wqT[:, cs], wkT[:, cs], wvT[:, cs][:, perm]], axis=1)),
            "wc": None,  # filled by caller (needs Wc)
            "bq": bq_arr,
            "bk": bk_arr,
            "vb1": vb1,
            "masks": masks,
        })
    return in_maps


_NC_CACHE = {}


def kernel(x, Wq, bq, Wk, bk, Wv, bv, Wc, bc):
    x = np.asarray(x, dtype=np.float32)
    T = x.shape[1]
    key = T
    if key not in _NC_CACHE:
        _NC_CACHE[key] = build_nc(T=T)
    nc = _NC_CACHE[key]

    in_maps = make_in_maps(x, Wq, bq, Wk, bk, Wv, bv, T=T)
    wcT = np.ascontiguousarray(np.asarray(Wc, np.float32).T).astype(
        ml_dtypes.bfloat16)  # [cin, cout]
    for core in range(N_CORES):
        hh = core % 2
        in_maps[core]["wc"] = np.ascontiguousarray(wcT[hh * CH:(hh + 1) * CH, :])

    res = run_bass_kernel_spmd(nc, in_maps, core_ids=list(range(N_CORES)))

    bc = np.asarray(bc, np.float32)
    out = np.empty((B, T, N_EMBD), np.float32)
    for b in range(B):
        out[b] = res.results[2 * b]["out"] + res.results[2 * b + 1]["out"] + bc
    return out


# revision 36
# speedup vs baseline: 1.3439x; 1.0209x over previous
"""Causal self-attention kernel for 8 TRN2 NeuronCores.

Sharding: 8 cores = 4 batches x 2 head-groups (8 heads / 512 channels each).
Each core computes q/k/v projections for its head half, causal attention for
its 8 heads, and a partial c_proj contracted over its 512 channels. The host
sums the two partials per batch and adds the c_proj bias.

All matmuls run in bf16 with fp32 PSUM accumulation. Host pre-transposes
x and the weight matrices so the device only ever does natural-layout DMAs.

Device layouts (per core):
  xt_sb [128, 8, T]        x^T tiles: [cin%128, cin//128, t]
  qt/kt_sb [128, 4, T]     Q^T/K^T: [d%128, head-pair, t] (head 2p: rows 0-63)
  v1_sb [128, T//128, 65, 8]  V interleaved [t%128, t//128, j, h]; col j<64 is
                           V_h[d=j] (host permutes Wv cols to dd*8+h), j=64
                           is the ones column used to accumulate the softmax
                           denominator inside the P@V matmul (M=65).

Scores are computed transposed (S^T[k, q]) so softmax'd probabilities feed
P@V directly as the moving operand; exp runs on ScalarE from PSUM with the
1/sqrt(hd) scale folded in; diagonal blocks are masked post-exp and trimmed
to their valid q-range. The attention inner loop is ScalarE-paced, so
projection / c_proj matmul groups are interleaved into it at k-tile
granularity to keep TensorE busy, and P@V is emitted one k-tile behind the
exp that feeds it.
"""

import numpy as np
import ml_dtypes
from contextlib import ExitStack

import concourse.bass as bass
import concourse.tile as tile
from concourse import bacc, mybir
from concourse.bass_utils import run_bass_kernel_spmd

BF16 = mybir.dt.bfloat16
F32 = mybir.dt.float32

N_EMBD = 1024
N_HEAD = 16
B = 4
T_FULL = 2048
HD = 64           # head dim
HPC = 8           # heads per core
CH = HPC * HD     # channels per core = 512
N_CORES = 8
SCALE = 1.0 / 8.0  # 1/sqrt(HD)

P = 128           # partitions
QC = 512          # q-chunk (matmul free dim)


def build_nc(T=T_FULL, pack_qk=True):
    """Build the per-core Bass module (same program on every core)."""
    n_tt = T // P          # 128-row tiles along T
    n_qc = T // QC         # 512-wide chunks along T
    n_ci = N_EMBD // P     # contraction tiles over the full embed dim
    n_dt = CH // P         # d-tiles of this core's 512 channels (= head pairs)
    n_oc = N_EMBD // QC

    nc = bacc.Bacc("TRN2", target_bir_lowering=False, debug=False)

    xt = nc.dram_tensor("xt", [N_EMBD, T], BF16, kind="ExternalInput").ap()
    wqkv = nc.dram_tensor("wqkv", [N_EMBD, 3 * CH], BF16,
                          kind="ExternalInput").ap()
    wc = nc.dram_tensor("wc", [CH, N_EMBD], BF16, kind="ExternalInput").ap()
    bq = nc.dram_tensor("bq", [P, n_dt], F32, kind="ExternalInput").ap()
    bk = nc.dram_tensor("bk", [P, n_dt], F32, kind="ExternalInput").ap()
    vb1 = nc.dram_tensor("vb1", [P, CH + HPC], F32, kind="ExternalInput").ap()
    masks = nc.dram_tensor("masks", [P, 4 * QC], BF16, kind="ExternalInput").ap()
    out = nc.dram_tensor("out", [T, N_EMBD], F32, kind="ExternalOutput").ap()

    with tile.TileContext(nc) as tc, ExitStack() as ctx:
        singles = ctx.enter_context(tc.tile_pool(name="singles", bufs=1))
        mm_ps = ctx.enter_context(tc.tile_pool(name="mm_ps", bufs=2, space="PSUM"))
        qk_ps_pool = ctx.enter_context(tc.tile_pool(name="qk_ps", bufs=3, space="PSUM"))
        av_ps_pool = ctx.enter_context(tc.tile_pool(name="av_ps", bufs=3, space="PSUM"))
        pt_pool = ctx.enter_context(tc.tile_pool(name="pt", bufs=12))
        small = ctx.enter_context(tc.tile_pool(name="small", bufs=4))
        dram = ctx.enter_context(tc.tile_pool(name="dram", bufs=4, space="DRAM"))
        ost = ctx.enter_context(tc.tile_pool(name="ost", bufs=3))

        # ---- resident tensors (split DMAs so compute starts early) ----
        xt_sb = singles.tile([P, n_ci, T], BF16)
        wqkv_sb = singles.tile([P, n_ci, 3 * CH], BF16)
        wq_sb = wqkv_sb[:, :, 0:CH]
        wk_sb = wqkv_sb[:, :, CH:2 * CH]
        wv_sb = wqkv_sb[:, :, 2 * CH:3 * CH]
        bq_sb = singles.tile([P, n_dt], F32)
        nc.sync.dma_start(bq_sb, bq)
        bk_sb = singles.tile([P, n_dt], F32)
        nc.sync.dma_start(bk_sb, bk)
        vb1_sb = singles.tile([P, CH + HPC], F32)
        nc.sync.dma_start(vb1_sb, vb1)
        mask_sb = singles.tile([P, 4, QC], BF16)
        nc.sync.dma_start(mask_sb, masks.rearrange("p (r q) -> p r q", r=4))
        for ci in range(n_ci):
            rows = slice(ci * P, (ci + 1) * P)
            nc.sync.dma_start(xt_sb[:, ci, :], xt[rows, :])
            nc.sync.dma_start(wqkv_sb[:, ci, :], wqkv[rows, :])
        wc_sb = singles.tile([P, n_dt, N_EMBD], BF16)
        for pr in range(n_dt):
            nc.sync.dma_start(wc_sb[:, pr, :], wc[pr * P:(pr + 1) * P, :])

        qt_sb = singles.tile([P, n_dt, T], BF16)   # [d%128, head-pair, t]
        kt_sb = singles.tile([P, n_dt, T], BF16)
        v1_sb = singles.tile([P, n_tt, HD + 1, HPC], BF16)
        yt_sb = singles.tile([P, n_dt, T], BF16)   # attention out (normalized)

        # ones column of v1 (written once, before any V tile is consumed)
        nc.vector.tensor_copy(
            out=v1_sb[:, :, HD, :],
            in_=vb1_sb[:, None, CH:].to_broadcast((P, n_tt, HPC)),
        )

        # ---- work units ----
        def emit_v(tt):
            ps = mm_ps.tile([P, CH], F32, tag="mm")
            for ci in range(n_ci):
                nc.tensor.matmul(
                    ps,
                    lhsT=xt_sb[:, ci, tt * P:(tt + 1) * P],
                    rhs=wv_sb[:, ci, :],
                    start=(ci == 0), stop=(ci == n_ci - 1),
                )
            nc.vector.tensor_add(
                out=v1_sb[:, tt, 0:HD, :],
                in0=ps.rearrange("p (j h) -> p j h", h=HPC),
                in1=vb1_sb[:, 0:CH].rearrange("p (j h) -> p j h", h=HPC),
            )

        def emit_proj(w_sb, b_sb, o_sb, pr, tcn):
            ps = mm_ps.tile([P, QC], F32, tag="mm")
            for ci in range(n_ci):
                nc.tensor.matmul(
                    ps,
                    lhsT=w_sb[:, ci, pr * P:(pr + 1) * P],
                    rhs=xt_sb[:, ci, tcn * QC:(tcn + 1) * QC],
                    start=(ci == 0), stop=(ci == n_ci - 1),
                )
            nc.vector.tensor_add(
                out=o_sb[:, pr, tcn * QC:(tcn + 1) * QC],
                in0=ps,
                in1=b_sb[:, pr, None].to_broadcast((P, QC)),
            )

        def emit_cproj(tt, oc, on_act=False):
            ps = mm_ps.tile([P, QC], F32, tag="mm")
            for pr in range(n_dt):
                nc.tensor.matmul(
                    ps,
                    lhsT=yt_sb[:, pr, tt * P:(tt + 1) * P],
                    rhs=wc_sb[:, pr, oc * QC:(oc + 1) * QC],
                    start=(pr == 0), stop=(pr == n_dt - 1),
                )
            st = ost.tile([P, QC], F32, tag="ost")
            if on_act:  # ScalarE is idle once attention's exp stream drains
                nc.scalar.activation(st, ps, mybir.ActivationFunctionType.Copy)
            else:
                nc.vector.tensor_copy(out=st, in_=ps)
            nc.sync.dma_start(
                out=out[tt * P:(tt + 1) * P, oc * QC:(oc + 1) * QC], in_=st,
            )

        # ---- fused pipeline over q-chunks ----
        # prologue: V for chunk 0 and Q/K projections for (pr=0, chunk 0)
        for tt in range(QC // P):
            emit_v(tt)
        emit_proj(wq_sb, bq_sb, qt_sb, 0, 0)
        emit_proj(wk_sb, bk_sb, kt_sb, 0, 0)

        for qc in range(n_qc):
            q0 = qc * QC
            nkt = (q0 + QC) // P  # causal: k-tiles 0..nkt-1

            # filler units: projections for chunk qc+1; all deferrable c_proj
            # is pushed into the last chunk, whose attention is ScalarE-bound
            # and leaves TensorE the most idle.
            fillers = []
            if qc + 1 < n_qc:
                for tt in range((qc + 1) * (QC // P), (qc + 2) * (QC // P)):
                    fillers.append(("v", tt))
                for pr in range(n_dt):
                    fillers.append(("q", pr, qc + 1))
                    fillers.append(("k", pr, qc + 1))
            if qc == n_qc - 1:
                for tt in range(0, (n_qc - 1) * (QC // P)):
                    for oc in range(n_oc):
                        fillers.append(("c", tt, oc))
            n_slots = n_dt * nkt
            per_slot = len(fillers) / n_slots
            if qc == n_qc - 1:
                per_slot *= 0.7  # hold filler back for the drain at chunk end
            facc = 0.0
            fi = 0

            for pr in range(n_dt):
                if qc == 0 and pr >= 1:
                    emit_proj(wq_sb, bq_sb, qt_sb, pr, 0)
                    emit_proj(wk_sb, bk_sb, kt_sb, pr, 0)
                av_A = av_ps_pool.tile([HD + 1, QC], F32, tag="av")
                av_B = av_ps_pool.tile([HD + 1, QC], F32, tag="av")
                pend = []  # delayed P@V queue: (kt, relq, pts)
                depth = 5
                for kt in range(nkt):
                    k0 = kt * P
                    rel = kt - (q0 // P)  # >=0 on diagonal block tiles
                    relq = rel * P if rel > 0 else 0
                    pts = []
                    for hi, prt in enumerate((slice(0, HD), slice(HD, P))):
                        qk = qk_ps_pool.tile([P, QC], F32, tag="qk")
                        nc.tensor.matmul(
                            qk[:, relq:],
                            lhsT=kt_sb[prt, pr, k0:k0 + P],
                            rhs=qt_sb[prt, pr, q0 + relq:q0 + QC],
                            start=True, stop=True,
                            tile_position=(prt.start, 0) if pack_qk else None,
                        )
                        pt = pt_pool.tile([P, QC], BF16, tag="pt")
                        nc.scalar.activation(
                            pt[:, relq:], qk[:, relq:],
                            mybir.ActivationFunctionType.Exp, scale=SCALE,
                        )
                        if rel >= 0:
                            nc.vector.tensor_mul(
                                pt[:, relq:], pt[:, relq:],
                                mask_sb[:, rel, relq:],
                            )
                        pts.append(pt)
                    pend.append((kt, relq, pts))
                    if len(pend) > depth:
                        pkt, prelq, ppts = pend.pop(0)
                        for hi, av in enumerate((av_A, av_B)):
                            nc.tensor.matmul(
                                av[:, prelq:],
                                lhsT=v1_sb[:, pkt, :, 2 * pr + hi],
                                rhs=ppts[hi][:, prelq:],
                                start=(pkt == 0), stop=False,
                            )
                    # interleave filler work to keep TensorE fed
                    facc += per_slot
                    while facc >= 1.0 and fi < len(fillers):
                        f = fillers[fi]
                        fi += 1
                        facc -= 1.0
                        if f[0] == "v":
                            emit_v(f[1])
                        elif f[0] == "q":
                            emit_proj(wq_sb, bq_sb, qt_sb, f[1], f[2])
                        elif f[0] == "k":
                            emit_proj(wk_sb, bk_sb, kt_sb, f[1], f[2])
                        else:
                            emit_cproj(f[1], f[2])
                # drain delayed P@V queue
                while pend:
                    pkt, prelq, ppts = pend.pop(0)
                    for hi, av in enumerate((av_A, av_B)):
                        nc.tensor.matmul(
                            av[:, prelq:],
                            lhsT=v1_sb[:, pkt, :, 2 * pr + hi],
                            rhs=ppts[hi][:, prelq:],
                            start=(pkt == 0), stop=(pkt == nkt - 1),
                        )
                # normalize: y[d, q] = av[d, q] / av[HD, q]; one DRAM bounce
                # broadcasts both heads' fp32 reciprocal rows across partitions
                r_sb = small.tile([HD + 1, 2, QC], F32, tag="recip")
                nc.vector.reciprocal(out=r_sb[HD:HD + 1, 0, :],
                                     in_=av_A[HD:HD + 1, :])
                nc.vector.reciprocal(out=r_sb[HD:HD + 1, 1, :],
                                     in_=av_B[HD:HD + 1, :])
                dscr = dram.tile([1, 2 * QC], F32, tag="dbounce")
                nc.gpsimd.dma_start(
                    out=dscr, in_=r_sb[HD:HD + 1].rearrange("p a q -> p (a q)"))
                den_sb = small.tile([HD, 2, QC], F32, tag="den")
                nc.gpsimd.dma_start(
                    out=den_sb,
                    in_=bass.AP(tensor=dscr.tensor, offset=dscr.offset,
                                ap=[[0, HD], [QC, 2], [1, QC]]),
                )
                nc.vector.tensor_mul(
                    out=yt_sb[0:HD, pr, q0:q0 + QC],
                    in0=av_A[0:HD, :], in1=den_sb[:, 0, :],
                )
                tmp = small.tile([HD, QC], BF16, tag="ytmp")
                nc.vector.tensor_mul(out=tmp, in0=av_B[0:HD, :],
                                     in1=den_sb[:, 1, :])
                nc.sync.dma_start(out=yt_sb[HD:P, pr, q0:q0 + QC], in_=tmp)

                if qc == n_qc - 1:
                    # drain held-back filler while the next pair's exps queue
                    for _ in range(2):
                        if fi < len(fillers):
                            f = fillers[fi]
                            fi += 1
                            if f[0] == "c":
                                emit_cproj(f[1], f[2])

            # any leftover fillers for this stage
            while fi < len(fillers):
                f = fillers[fi]
                fi += 1
                if f[0] == "v":
                    emit_v(f[1])
                elif f[0] == "q":
                    emit_proj(wq_sb, bq_sb, qt_sb, f[1], f[2])
                elif f[0] == "k":
                    emit_proj(wk_sb, bk_sb, kt_sb, f[1], f[2])
                else:
                    emit_cproj(f[1], f[2])

        # epilogue: c_proj for the last chunk (stage copies on idle ScalarE)
        for tt in range((n_qc - 1) * (QC // P), n_qc * (QC // P)):
            for oc in range(n_oc):
                emit_cproj(tt, oc, on_act=True)

    nc.compile()
    return nc


def make_in_maps(x, Wq, bq, Wk, bk, Wv, bv, T=T_FULL):
    """Host-side sharding + layout prep. Returns per-core input dicts."""
    bf = ml_dtypes.bfloat16
    x = np.asarray(x, dtype=np.float32)
    n_dt = CH // P

    # causal masks for the 4 diagonal-relative offsets
    k_idx = np.arange(P)[:, None]
    q_idx = np.arange(QC)[None, :]
    masks = np.concatenate(
        [(r * P + k_idx <= q_idx) for r in range(4)], axis=1
    ).astype(bf)  # [128, 4*512] packed along the free dim

    # head-interleave permutation for Wv columns: new col j*HPC+h = old h*HD+j
    j = np.arange(HD)[:, None]
    h = np.arange(HPC)[None, :]
    perm = (h * HD + j).reshape(-1)  # new[j*HPC+h] <- old[h*HD+j]

    wqT = np.ascontiguousarray(Wq.T).astype(bf)  # [cin, dout]
    wkT = np.ascontiguousarray(Wk.T).astype(bf)
    wvT = np.ascontiguousarray(Wv.T).astype(bf)

    in_maps = []
    for core in range(N_CORES):
        b = core // 2
        hh = core % 2
        cs = slice(hh * CH, (hh + 1) * CH)
        xtb = np.ascontiguousarray(x[b, :T].T).astype(bf)  # [N_EMBD, T]

        bq_arr = np.asarray(bq[cs], np.float32).reshape(n_dt, P).T.copy()
        bk_arr = np.asarray(bk[cs], np.float32).reshape(n_dt, P).T.copy()
        bv_half = np.asarray(bv[cs], np.float32)
        vb1 = np.concatenate([bv_half[perm], np.ones(HPC, np.float32)])
        vb1 = np.broadcast_to(vb1, (P, CH + HPC)).copy()

        in_maps.append({
            "xt": xtb,
            "wqkv": np.ascontiguousarray(np.concatenate(
                [wqT[:, cs], wkT[:, cs], wvT[:, cs][:, perm]], axis=1)),
            "wc": None,  # filled by caller (needs Wc)
            "bq": bq_arr,
            "bk": bk_arr,
            "vb1": vb1,
            "masks": masks,
        })
    return in_maps


_NC_CACHE = {}


def kernel(x, Wq, bq, Wk, bk, Wv, bv, Wc, bc):
    x = np.asarray(x, dtype=np.float32)
    T = x.shape[1]
    key = T
    if key not in _NC_CACHE:
        _NC_CACHE[key] = build_nc(T=T)
    nc = _NC_CACHE[key]

    in_maps = make_in_maps(x, Wq, bq, Wk, bk, Wv, bv, T=T)
    wcT = np.ascontiguousarray(np.asarray(Wc, np.float32).T).astype(
        ml_dtypes.bfloat16)  # [cin, cout]
    for core in range(N_CORES):
        hh = core % 2
        in_maps[core]["wc"] = np.ascontiguousarray(wcT[hh * CH:(hh + 1) * CH, :])

    res = run_bass_kernel_spmd(nc, in_maps, core_ids=list(range(N_CORES)))

    bc = np.asarray(bc, np.float32)
    out = np.empty((B, T, N_EMBD), np.float32)
    for b in range(B):
        out[b] = res.results[2 * b]["out"] + res.results[2 * b + 1]["out"] + bc
    return out
